# revision 22
# baseline (speedup 1.0000x reference)
"""Paged GQA attention (diffusion block-causal) on 8 TRN2 NeuronCores.

Problem: B=8 seqs x LQ=128 new tokens, 32 q heads / 8 kv heads, head_dim 128,
ctx_len=2048 cached tokens per seq (paged KV cache, 16-token pages), plus the
128 new tokens; block-causal mask (block 32) over the new-token region.

Sharding: one sequence per NeuronCore (8 seqs -> 8 cores), no collectives.

Per-core device kernel, per kv-head group g (4 q heads share a kv head):
  scoresT[k, q4] = K_g^T-tile.T @ Q_g          (fp32r matmul, N=512 full rate)
  probsT = exp(scoresT * scale + mask)         (ACT, bf16 out; mask on last tile)
  outU[q, d+1] += probsT_head.T @ [V_g | 1]    (bf16 matmul; last col = sum)
  out[q, d] = outU[:, :d] * (1 / outU[:, d])   (DVE reciprocal + tensor_scalar)

Host side: gather pages, build K^T / Q^T / V-augmented layouts, shard, gather.
"""

import sys

if '/opt/trn_rl_repo' not in sys.path:
    sys.path.insert(0, '/opt/trn_rl_repo')

import math

import ml_dtypes
import numpy as np

B = 8
LQ = 128
NH = 32
NKV = 8
GROUP = NH // NKV  # 4
HD = 128
PAGE = 16
CTX = 2048
K_TOT = CTX + LQ          # 2176
KT_TILES = K_TOT // 128   # 17
QG = GROUP * LQ           # 512 (4 heads x 128 queries)
SCALE = 1.0 / math.sqrt(HD)
MASK_NEG = -30000.0

_CACHE = {}


def _build_nc():
    import concourse.bass as bass
    import concourse.mybir as mybir
    from concourse.tile import TileContext
    from concourse.vector_clock import ScopedClock

    class TileContextP(TileContext):
        """TileContext adapted to this walrus build, which only supports ONE
        sync-wait per instruction: extra waits are hoisted onto same-engine
        NoOps emitted immediately before the instruction."""

        def _commit_instruction(self, inst, lazy_reg_writes=True):
            si = getattr(inst, "sync_info", None)
            eng = getattr(inst, "engine", None)
            if si is not None and eng is not None:
                waits = list(si.on_wait or [])
                if len(waits) > 1:
                    for w in waits[:-1]:
                        nop = mybir.InstNoOp(
                            name=self.nc.get_next_instruction_name(),
                            sync_info=mybir.SyncInfo(on_wait=[w], on_update=[]),
                            bass_nofuse=True,
                            engine=eng,
                        )
                        super()._commit_instruction(nop, lazy_reg_writes=False)
                    si.on_wait = [waits[-1]]
            return super()._commit_instruction(inst, lazy_reg_writes)

        def _drain_and_barrier(self, tick_clock, wait_clock):
            nc = self.nc
            drain_inst = nc.sync.drain()
            wait_clock.add_sem_waits(
                drain_inst.ins, ScopedClock({None: tick_clock.global_clock})
            )
            si = drain_inst.ins.sync_info
            waits = list(si.on_wait or []) if si is not None else []
            if len(waits) > 1:
                si.on_wait = [waits[0]]
                # distribute remaining waits round-robin across engines so
                # they resolve in parallel rather than serially on SP
                engs = [nc.vector, nc.scalar, nc.tensor, nc.gpsimd, nc.sync]
                for j, w in enumerate(waits[1:]):
                    d = engs[j % len(engs)].drain()
                    d.ins.sync_info = mybir.SyncInfo(on_wait=[w], on_update=[])
            nc.all_engine_barrier()
            assert self.sems is not None
            popped = nc._tile_sem_poison_stack.pop()
            assert popped is self._sem_poison
            nc.clear_and_free_semaphores(list(self.sems.allocated().values()))
            nc.all_engine_barrier()

    f32 = mybir.dt.float32
    bf16 = mybir.dt.bfloat16

    nc = bass.Bass("TRN2")
    qt = nc.dram_tensor("qt", [NKV, HD, QG], bf16, kind="ExternalInput")
    kt = nc.dram_tensor("kt", [NKV, HD, K_TOT], bf16, kind="ExternalInput")
    va = nc.dram_tensor("va", [NKV, KT_TILES, 128, HD + 1], bf16,
                        kind="ExternalInput")
    mk = nc.dram_tensor("mk", [128, QG], bf16, kind="ExternalInput")
    o = nc.dram_tensor("o", [LQ, NH * HD], f32, kind="ExternalOutput")

    # k-tiles are exp'd in triples: one [128, 3*QG] PSUM chunk spans three
    # banks, cutting the per-ACTIVATE fixed cost. 17 tiles -> 5 triples + 1
    # pair. PSUM: scores 3 banks x 2 bufs + 2 packed accumulator banks = 8.
    chunks = [(3 * i, 3) for i in range(5)] + [(15, 2)]

    with TileContextP(nc) as tc:
        with (
            tc.tile_pool(name="kp", bufs=NKV) as kp,
            tc.tile_pool(name="vp", bufs=NKV) as vp,
            tc.tile_pool(name="qp", bufs=NKV) as qp,
            tc.tile_pool(name="mp", bufs=1) as mp,
            tc.tile_pool(name="pp", bufs=3) as pp,
            tc.tile_pool(name="rp", bufs=8) as rp,
            tc.tile_pool(name="ob", bufs=2) as ob,
            tc.tile_pool(name="sp", bufs=2, space="PSUM") as sp,
            tc.tile_pool(name="op", bufs=2, space="PSUM") as op,
        ):
            # warm the ACT exp table while the first DMAs are in flight
            warm = mp.tile([128, 1], mybir.dt.float32, name="warm")
            nc.vector.memset(warm, 0.0)
            nc.scalar.activation(warm, warm,
                                 mybir.ActivationFunctionType.Exp)

            mask_sb = mp.tile([128, QG], bf16)
            nc.sync.dma_start(out=mask_sb, in_=mk[:, :])

            # keep the PE busy through the DMA-bound startup so the HAM
            # clock gate is released (2.4 GHz) before the real matmuls
            wsrc = mp.tile([128, 128], bf16, name="wsrc")
            nc.vector.memset(wsrc, 0.0)
            wps = sp.tile([128, 3 * QG], mybir.dt.float32, tag="s",
                          name="wps")
            for _ in range(20):
                nc.tensor.matmul(wps[:, 0:128], lhsT=wsrc, rhs=wsrc,
                                 start=True, stop=True, skip_group_check=True)

            # Flat software pipeline over (group, chunk): each chunk's PV
            # matmuls are emitted AFTER the next chunk's QK matmuls so the
            # in-order PE stream never head-of-line blocks on exp results or
            # on the previous group's accumulator release.
            accs_of = {}
            qt_of, kt_of, va_of = {}, {}, {}

            def load_group(g):
                qt_sb = qp.tile([HD, QG], bf16, tag="qt", name=f"qt{g}")
                nc.sync.dma_start(out=qt_sb, in_=qt[g])
                kt_sb = kp.tile([HD, K_TOT], bf16, tag="kt", name=f"kt{g}")
                # split so the first chunk's K columns land early
                nc.sync.dma_start(out=kt_sb[:, :3 * 128],
                                  in_=kt[g][:, :3 * 128])
                nc.sync.dma_start(out=kt_sb[:, 3 * 128:],
                                  in_=kt[g][:, 3 * 128:])
                va_sb = vp.tile([128, KT_TILES, HD + 1], bf16, tag="va",
                                name=f"va{g}")
                nc.scalar.dma_start(
                    out=va_sb, in_=va[g].rearrange("t p c -> p t c"))
                kt_of[g], va_of[g], qt_of[g] = kt_sb, va_sb, qt_sb
                # two heads' [q, d+1] accumulators packed per PSUM bank
                accs_of[g] = [
                    op.tile([LQ, 2, HD + 1], mybir.dt.float32, tag="acc",
                            name=f"acc_{g}_{p}")
                    for p in range(GROUP // 2)
                ]

            def emit_qk_exp(g, t0, width):
                s_ps = sp.tile([128, 3 * QG], mybir.dt.float32, tag="s",
                               name=f"s_{g}_{t0}")
                for tt in range(width):
                    nc.tensor.matmul(
                        s_ps[:, tt * QG:(tt + 1) * QG],
                        lhsT=kt_of[g][:, (t0 + tt) * 128:(t0 + tt + 1) * 128],
                        rhs=qt_of[g],
                        start=True,
                        stop=True,
                    )
                p_sb = pp.tile([128, 3 * QG], bf16, tag="p",
                               name=f"p_{g}_{t0}")
                nc.scalar.activation(
                    p_sb[:, :width * QG], s_ps[:, :width * QG],
                    mybir.ActivationFunctionType.Exp, scale=SCALE,
                )
                if t0 + width == KT_TILES:
                    # new-token tile: multiplicative block-causal mask applied
                    # post-exp on DVE (keeps ACT's critical path mask-free)
                    lo = (width - 1) * QG
                    nc.vector.tensor_mul(
                        p_sb[:, lo:lo + QG], p_sb[:, lo:lo + QG], mask_sb)
                return p_sb

            def emit_pv(g, t0, width, p_sb):
                for tt in range(width):
                    t = t0 + tt
                    for h in range(GROUP):
                        # start=True clears has_written for the WHOLE bank, so
                        # only the first head sharing the bank issues it; the
                        # second head's t=0 write lands on has_written=0 and
                        # overwrites rather than accumulates.
                        nc.tensor.matmul(
                            accs_of[g][h // 2][:, h % 2, :],
                            lhsT=p_sb[:, tt * QG + h * LQ:
                                      tt * QG + (h + 1) * LQ],
                            rhs=va_of[g][:, t, :],
                            start=(t == 0 and h % 2 == 0),
                            stop=(t == KT_TILES - 1),
                            skip_group_check=True,
                        )

            def emit_normalize(g):
                o_sb = ob.tile([128, GROUP * HD], f32, tag="osb",
                               name=f"osb{g}")
                for h in range(GROUP):
                    acc = accs_of[g][h // 2][:, h % 2, :]
                    rec = rp.tile([LQ, 1], mybir.dt.float32, tag="rec",
                                  name=f"rec_{g}_{h}")
                    nc.vector.reciprocal(rec, acc[:, HD:HD + 1])
                    nc.vector.tensor_scalar_mul(
                        o_sb[:, h * HD:(h + 1) * HD],
                        acc[:, 0:HD],
                        rec,
                    )
                nc.sync.dma_start(
                    out=o[:, g * GROUP * HD:(g + 1) * GROUP * HD], in_=o_sb)

            work = [(g, t0, w) for g in range(NKV) for t0, w in chunks]
            for g in range(NKV):
                load_group(g)
            pending = None  # (g, t0, width, p_sb) awaiting PV emission
            for g, t0, w in work:
                p_sb = emit_qk_exp(g, t0, w)
                if pending is not None:
                    emit_pv(*pending)
                    if pending[1] + pending[2] == KT_TILES:
                        emit_normalize(pending[0])
                pending = (g, t0, w, p_sb)
            emit_pv(*pending)
            emit_normalize(pending[0])
    return nc


def _prep_inputs(q, k, v, k_cache, v_cache, page_tables, ctx_len, block_size):
    ctx = int(ctx_len)
    bs = int(block_size)
    assert ctx == CTX, f"kernel compiled for ctx_len={CTX}, got {ctx}"
    npages = ctx // PAGE

    q = np.asarray(q, np.float32).reshape(B, LQ, NH, HD)
    k = np.asarray(k, np.float32).reshape(B, LQ, NKV, HD)
    v = np.asarray(v, np.float32).reshape(B, LQ, NKV, HD)
    k_cache = np.asarray(k_cache, np.float32)
    v_cache = np.asarray(v_cache, np.float32)
    pt = np.asarray(page_tables).astype(np.int64)[:, :npages]

    # paged gather: [B, ctx, NKV, HD]
    k_ctx = k_cache[pt].reshape(B, ctx, NKV, HD)
    v_ctx = v_cache[pt].reshape(B, ctx, NKV, HD)
    k_full = np.concatenate([k_ctx, k], axis=1)   # [B, K_TOT, NKV, HD]
    v_full = np.concatenate([v_ctx, v], axis=1)

    # K^T per core: [NKV, HD, K_TOT], bf16
    kt = np.ascontiguousarray(
        k_full.transpose(0, 2, 3, 1)).astype(ml_dtypes.bfloat16)
    # V augmented with ones column, bf16: [B, NKV, K_TOT, HD+1]
    v_t = v_full.transpose(0, 2, 1, 3)            # [B, NKV, K_TOT, HD]
    va = np.empty((B, NKV, K_TOT, HD + 1), np.float32)
    va[..., :HD] = v_t
    va[..., HD] = 1.0
    va = va.astype(ml_dtypes.bfloat16).reshape(B, NKV, KT_TILES, 128, HD + 1)
    # Q^T per group: [B, NKV, HD, GROUP*LQ]
    qh = q.transpose(0, 2, 3, 1).reshape(B, NKV, GROUP, HD, LQ)
    qt = np.ascontiguousarray(qh.transpose(0, 1, 3, 2, 4)).reshape(
        B, NKV, HD, QG).astype(ml_dtypes.bfloat16)

    # multiplicative 0/1 mask on the new-token k-tile, scoresT coords [k, q4]
    kj = np.arange(128)
    qi = np.arange(LQ)
    allowed = (kj[:, None] // bs) <= (qi[None, :] // bs)   # [128, LQ]
    mrow = allowed.astype(np.float32)
    mask = np.tile(mrow, (1, GROUP)).astype(ml_dtypes.bfloat16)  # [128, QG]

    in_maps = []
    for b in range(B):
        in_maps.append({
            "qt": qt[b],
            "kt": kt[b],
            "va": va[b],
            "mk": mask,
        })
    return in_maps


def _run(inputs, trace=False):
    from concourse.bass_utils import run_bass_kernel_spmd

    if "nc" not in _CACHE:
        _CACHE["nc"] = _build_nc()
    nc = _CACHE["nc"]
    in_maps = _prep_inputs(**inputs)
    res = run_bass_kernel_spmd(
        nc, in_maps, core_ids=list(range(B)), trace=trace,
    )
    out = np.empty((B * LQ, NH * HD), np.float32)
    for b in range(B):
        out[b * LQ:(b + 1) * LQ] = res.results[b]["o"]
    return out, res


def kernel(**inputs):
    out, _ = _run(inputs, trace=False)
    return out


# revision 25
# speedup vs baseline: 1.2465x; 1.2465x over previous
"""Paged GQA attention (diffusion block-causal) on 8 TRN2 NeuronCores.

Problem: B=8 seqs x LQ=128 new tokens, 32 q heads / 8 kv heads, head_dim 128,
ctx_len=2048 cached tokens per seq (paged KV cache, 16-token pages), plus the
128 new tokens; block-causal mask (block 32) over the new-token region.

Sharding: one sequence per NeuronCore (8 seqs -> 8 cores), no collectives.

Per-core device kernel, per kv-head group g (4 q heads share a kv head):
  scoresT[k, q4] = K_g^T-tile.T @ Q_g          (fp32r matmul, N=512 full rate)
  probsT = exp(scoresT * scale + mask)         (ACT, bf16 out; mask on last tile)
  outU[q, d+1] += probsT_head.T @ [V_g | 1]    (bf16 matmul; last col = sum)
  out[q, d] = outU[:, :d] * (1 / outU[:, d])   (DVE reciprocal + tensor_scalar)

Host side: gather pages, build K^T / Q^T / V-augmented layouts, shard, gather.
"""

import sys

if '/opt/trn_rl_repo' not in sys.path:
    sys.path.insert(0, '/opt/trn_rl_repo')

import math

import ml_dtypes
import numpy as np

B = 8
LQ = 128
NH = 32
NKV = 8
GROUP = NH // NKV  # 4
HD = 128
PAGE = 16
CTX = 2048
K_TOT = CTX + LQ          # 2176
KT_TILES = K_TOT // 128   # 17
QG = GROUP * LQ           # 512 (4 heads x 128 queries)
SCALE = 1.0 / math.sqrt(HD)
MASK_NEG = -30000.0

_CACHE = {}


def _build_nc():
    import concourse.bass as bass
    import concourse.mybir as mybir
    from concourse.tile import TileContext
    from concourse.vector_clock import ScopedClock

    class TileContextP(TileContext):
        """TileContext adapted to this walrus build, which only supports ONE
        sync-wait per instruction: extra waits are hoisted onto same-engine
        NoOps emitted immediately before the instruction."""

        def _commit_instruction(self, inst, lazy_reg_writes=True):
            si = getattr(inst, "sync_info", None)
            eng = getattr(inst, "engine", None)
            if si is not None and eng is not None:
                waits = list(si.on_wait or [])
                if len(waits) > 1:
                    for w in waits[:-1]:
                        nop = mybir.InstNoOp(
                            name=self.nc.get_next_instruction_name(),
                            sync_info=mybir.SyncInfo(on_wait=[w], on_update=[]),
                            bass_nofuse=True,
                            engine=eng,
                        )
                        super()._commit_instruction(nop, lazy_reg_writes=False)
                    si.on_wait = [waits[-1]]
            return super()._commit_instruction(inst, lazy_reg_writes)

        def _drain_and_barrier(self, tick_clock, wait_clock):
            nc = self.nc
            drain_inst = nc.sync.drain()
            wait_clock.add_sem_waits(
                drain_inst.ins, ScopedClock({None: tick_clock.global_clock})
            )
            si = drain_inst.ins.sync_info
            waits = list(si.on_wait or []) if si is not None else []
            if len(waits) > 1:
                si.on_wait = [waits[0]]
                # distribute remaining waits round-robin across engines so
                # they resolve in parallel rather than serially on SP
                engs = [nc.vector, nc.scalar, nc.tensor, nc.gpsimd, nc.sync]
                for j, w in enumerate(waits[1:]):
                    d = engs[j % len(engs)].drain()
                    d.ins.sync_info = mybir.SyncInfo(on_wait=[w], on_update=[])
            nc.all_engine_barrier()
            assert self.sems is not None
            popped = nc._tile_sem_poison_stack.pop()
            assert popped is self._sem_poison
            nc.clear_and_free_semaphores(list(self.sems.allocated().values()))
            nc.all_engine_barrier()

    f32 = mybir.dt.float32
    bf16 = mybir.dt.bfloat16

    nc = bass.Bass("TRN2")
    qt = nc.dram_tensor("qt", [NKV, HD, QG], bf16, kind="ExternalInput")
    kt = nc.dram_tensor("kt", [NKV, HD, K_TOT], bf16, kind="ExternalInput")
    va = nc.dram_tensor("va", [NKV, KT_TILES, 128, HD + 1], bf16,
                        kind="ExternalInput")
    mk = nc.dram_tensor("mk", [128, QG], bf16, kind="ExternalInput")
    o = nc.dram_tensor("o", [LQ, NH * HD], f32, kind="ExternalOutput")

    # k-tiles are exp'd in triples: one [128, 3*QG] PSUM chunk spans three
    # banks, cutting the per-ACTIVATE fixed cost. 17 tiles -> 5 triples + 1
    # pair. PSUM: scores 3 banks x 2 bufs + 2 packed accumulator banks = 8.
    chunks = [(3 * i, 3) for i in range(5)] + [(15, 2)]

    with TileContextP(nc) as tc:
        with (
            tc.tile_pool(name="kp", bufs=2) as kp,
            tc.tile_pool(name="vp", bufs=2) as vp,
            tc.tile_pool(name="qp", bufs=2) as qp,
            tc.tile_pool(name="mp", bufs=1) as mp,
            tc.tile_pool(name="pp", bufs=3) as pp,
            tc.tile_pool(name="rp", bufs=8) as rp,
            tc.tile_pool(name="ob", bufs=2) as ob,
            tc.tile_pool(name="sp", bufs=2, space="PSUM") as sp,
            tc.tile_pool(name="op", bufs=2, space="PSUM") as op,
        ):
            # warm the ACT exp table while the first DMAs are in flight
            warm = mp.tile([128, 1], mybir.dt.float32, name="warm")
            nc.vector.memset(warm, 0.0)
            nc.scalar.activation(warm, warm,
                                 mybir.ActivationFunctionType.Exp)

            mask_sb = mp.tile([128, QG], bf16)
            nc.sync.dma_start(out=mask_sb, in_=mk[:, :])

            # keep the PE busy through the DMA-bound startup so the HAM
            # clock gate is released (2.4 GHz) before the real matmuls
            wsrc = mp.tile([128, 128], bf16, name="wsrc")
            nc.vector.memset(wsrc, 0.0)
            wps = sp.tile([128, 3 * QG], mybir.dt.float32, tag="s",
                          name="wps")
            for _ in range(30):
                nc.tensor.matmul(wps[:, 0:128], lhsT=wsrc, rhs=wsrc,
                                 start=True, stop=True, skip_group_check=True)

            # Flat software pipeline over (group, chunk): each chunk's PV
            # matmuls are emitted AFTER the next chunk's QK matmuls so the
            # in-order PE stream never head-of-line blocks on exp results or
            # on the previous group's accumulator release.
            accs_of = {}
            qt_of, kt_of, va_of = {}, {}, {}

            def load_group(g):
                qt_sb = qp.tile([HD, QG], bf16, tag="qt", name=f"qt{g}")
                nc.sync.dma_start(out=qt_sb, in_=qt[g])
                kt_sb = kp.tile([HD, K_TOT], bf16, tag="kt", name=f"kt{g}")
                # split so the first chunk's K columns land early
                nc.sync.dma_start(out=kt_sb[:, :3 * 128],
                                  in_=kt[g][:, :3 * 128])
                nc.sync.dma_start(out=kt_sb[:, 3 * 128:],
                                  in_=kt[g][:, 3 * 128:])
                va_sb = vp.tile([128, KT_TILES, HD + 1], bf16, tag="va",
                                name=f"va{g}")
                nc.scalar.dma_start(
                    out=va_sb, in_=va[g].rearrange("t p c -> p t c"))
                kt_of[g], va_of[g], qt_of[g] = kt_sb, va_sb, qt_sb
                # two heads' [q, d+1] accumulators packed per PSUM bank
                accs_of[g] = [
                    op.tile([LQ, 2, HD + 1], mybir.dt.float32, tag="acc",
                            name=f"acc_{g}_{p}")
                    for p in range(GROUP // 2)
                ]

            def emit_qk_exp(g, t0, width):
                s_ps = sp.tile([128, 3 * QG], mybir.dt.float32, tag="s",
                               name=f"s_{g}_{t0}")
                for tt in range(width):
                    nc.tensor.matmul(
                        s_ps[:, tt * QG:(tt + 1) * QG],
                        lhsT=kt_of[g][:, (t0 + tt) * 128:(t0 + tt + 1) * 128],
                        rhs=qt_of[g],
                        start=True,
                        stop=True,
                    )
                p_sb = pp.tile([128, 3 * QG], bf16, tag="p",
                               name=f"p_{g}_{t0}")
                nc.scalar.activation(
                    p_sb[:, :width * QG], s_ps[:, :width * QG],
                    mybir.ActivationFunctionType.Exp, scale=SCALE,
                )
                if t0 + width == KT_TILES:
                    # new-token tile: multiplicative block-causal mask applied
                    # post-exp on DVE (keeps ACT's critical path mask-free)
                    lo = (width - 1) * QG
                    nc.vector.tensor_mul(
                        p_sb[:, lo:lo + QG], p_sb[:, lo:lo + QG], mask_sb)
                return p_sb

            def emit_pv(g, t0, width, p_sb):
                for tt in range(width):
                    t = t0 + tt
                    for h in range(GROUP):
                        # start=True clears has_written for the WHOLE bank, so
                        # only the first head sharing the bank issues it; the
                        # second head's t=0 write lands on has_written=0 and
                        # overwrites rather than accumulates.
                        nc.tensor.matmul(
                            accs_of[g][h // 2][:, h % 2, :],
                            lhsT=p_sb[:, tt * QG + h * LQ:
                                      tt * QG + (h + 1) * LQ],
                            rhs=va_of[g][:, t, :],
                            start=(t == 0 and h % 2 == 0),
                            stop=(t == KT_TILES - 1),
                            skip_group_check=True,
                        )

            def emit_normalize(g):
                o_sb = ob.tile([128, GROUP * HD], f32, tag="osb",
                               name=f"osb{g}")
                for h in range(GROUP):
                    acc = accs_of[g][h // 2][:, h % 2, :]
                    rec = rp.tile([LQ, 1], mybir.dt.float32, tag="rec",
                                  name=f"rec_{g}_{h}")
                    nc.vector.reciprocal(rec, acc[:, HD:HD + 1])
                    nc.vector.tensor_scalar_mul(
                        o_sb[:, h * HD:(h + 1) * HD],
                        acc[:, 0:HD],
                        rec,
                    )
                nc.sync.dma_start(
                    out=o[:, g * GROUP * HD:(g + 1) * GROUP * HD], in_=o_sb)

            work = [(g, t0, w) for g in range(NKV) for t0, w in chunks]
            load_group(0)
            pending = None  # (g, t0, width, p_sb) awaiting PV emission
            for g, t0, w in work:
                if t0 == 0 and g + 1 < NKV:
                    load_group(g + 1)
                p_sb = emit_qk_exp(g, t0, w)
                if pending is not None:
                    emit_pv(*pending)
                    if pending[1] + pending[2] == KT_TILES:
                        emit_normalize(pending[0])
                pending = (g, t0, w, p_sb)
            emit_pv(*pending)
            emit_normalize(pending[0])
    return nc


def _prep_inputs(q, k, v, k_cache, v_cache, page_tables, ctx_len, block_size):
    ctx = int(ctx_len)
    bs = int(block_size)
    assert ctx == CTX, f"kernel compiled for ctx_len={CTX}, got {ctx}"
    npages = ctx // PAGE

    q = np.asarray(q, np.float32).reshape(B, LQ, NH, HD)
    k = np.asarray(k, np.float32).reshape(B, LQ, NKV, HD)
    v = np.asarray(v, np.float32).reshape(B, LQ, NKV, HD)
    k_cache = np.asarray(k_cache, np.float32)
    v_cache = np.asarray(v_cache, np.float32)
    pt = np.asarray(page_tables).astype(np.int64)[:, :npages]

    # paged gather: [B, ctx, NKV, HD]
    k_ctx = k_cache[pt].reshape(B, ctx, NKV, HD)
    v_ctx = v_cache[pt].reshape(B, ctx, NKV, HD)
    k_full = np.concatenate([k_ctx, k], axis=1)   # [B, K_TOT, NKV, HD]
    v_full = np.concatenate([v_ctx, v], axis=1)

    # K^T per core: [NKV, HD, K_TOT], bf16
    kt = np.ascontiguousarray(
        k_full.transpose(0, 2, 3, 1)).astype(ml_dtypes.bfloat16)
    # V augmented with ones column, bf16: [B, NKV, K_TOT, HD+1]
    v_t = v_full.transpose(0, 2, 1, 3)            # [B, NKV, K_TOT, HD]
    va = np.empty((B, NKV, K_TOT, HD + 1), np.float32)
    va[..., :HD] = v_t
    va[..., HD] = 1.0
    va = va.astype(ml_dtypes.bfloat16).reshape(B, NKV, KT_TILES, 128, HD + 1)
    # Q^T per group: [B, NKV, HD, GROUP*LQ]
    qh = q.transpose(0, 2, 3, 1).reshape(B, NKV, GROUP, HD, LQ)
    qt = np.ascontiguousarray(qh.transpose(0, 1, 3, 2, 4)).reshape(
        B, NKV, HD, QG).astype(ml_dtypes.bfloat16)

    # multiplicative 0/1 mask on the new-token k-tile, scoresT coords [k, q4]
    kj = np.arange(128)
    qi = np.arange(LQ)
    allowed = (kj[:, None] // bs) <= (qi[None, :] // bs)   # [128, LQ]
    mrow = allowed.astype(np.float32)
    mask = np.tile(mrow, (1, GROUP)).astype(ml_dtypes.bfloat16)  # [128, QG]

    in_maps = []
    for b in range(B):
        in_maps.append({
            "qt": qt[b],
            "kt": kt[b],
            "va": va[b],
            "mk": mask,
        })
    return in_maps


def _run(inputs, trace=False):
    from concourse.bass_utils import run_bass_kernel_spmd

    if "nc" not in _CACHE:
        _CACHE["nc"] = _build_nc()
    nc = _CACHE["nc"]
    in_maps = _prep_inputs(**inputs)
    res = run_bass_kernel_spmd(
        nc, in_maps, core_ids=list(range(B)), trace=trace,
    )
    out = np.empty((B * LQ, NH * HD), np.float32)
    for b in range(B):
        out[b * LQ:(b + 1) * LQ] = res.results[b]["o"]
    return out, res


def kernel(**inputs):
    out, _ = _run(inputs, trace=False)
    return out


# revision 36
# speedup vs baseline: 1.2823x; 1.0288x over previous
"""Paged GQA attention (diffusion block-causal) on 8 TRN2 NeuronCores.

Problem: B=8 seqs x LQ=128 new tokens, 32 q heads / 8 kv heads, head_dim 128,
ctx_len=2048 cached tokens per seq (paged KV cache, 16-token pages), plus the
128 new tokens; block-causal mask (block 32) over the new-token region.

Sharding: one sequence per NeuronCore (8 seqs -> 8 cores), no collectives.

Per-core device kernel, per kv-head group g (4 q heads share a kv head):
  scoresT[k, q4] = K_g^T-tile.T @ Q_g          (fp32r matmul, N=512 full rate)
  probsT = exp(scoresT * scale + mask)         (ACT, bf16 out; mask on last tile)
  outU[q, d+1] += probsT_head.T @ [V_g | 1]    (bf16 matmul; last col = sum)
  out[q, d] = outU[:, :d] * (1 / outU[:, d])   (DVE reciprocal + tensor_scalar)

Host side: gather pages, build K^T / Q^T / V-augmented layouts, shard, gather.
"""

import sys

if '/opt/trn_rl_repo' not in sys.path:
    sys.path.insert(0, '/opt/trn_rl_repo')

import math

import ml_dtypes
import numpy as np

B = 8
LQ = 128
NH = 32
NKV = 8
GROUP = NH // NKV  # 4
HD = 128
PAGE = 16
CTX = 2048
K_TOT = CTX + LQ          # 2176
KT_TILES = K_TOT // 128   # 17
QG = GROUP * LQ           # 512 (4 heads x 128 queries)
SCALE = 1.0 / math.sqrt(HD)
MASK_NEG = -30000.0

_CACHE = {}


def _register_dve_exp_ops():
    """Two chained custom DVE ops computing exp(x*SCALE) at line rate on the
    Vector engine: p = deg-3 Taylor of e^(x*SCALE/32), then p^32 via five
    squarings. Lets DVE take a share of the softmax exp off the Scalar
    engine (the bottleneck)."""
    import numpy as np
    from concourse import dve_ops as D
    from concourse.dve_spec import (
        C0, C1, C2, One, Spec, Src0, _has_src1, lower, sq,
    )
    from concourse.dve_uop import DveOpSpec

    if "EXP32P_ANT" in D._SUB_OPCODE_FOR_NAME:
        by_name = {op.name: op for op in D.OPS}
        return by_name["EXP32P_ANT"], by_name["POW32_ANT"]

    v = Src0 * C0
    v2 = v * v
    body1 = (One + v) + v2 * (C1 + v * C2)

    def ref1(in0, in1, c0, c1, c2):
        vv = in0.astype(np.float32) * c0
        return (1.0 + vv + vv * vv * (c1 + vv * c2)).astype(np.float32)

    body2 = sq(sq(sq(sq(sq(Src0)))))

    def ref2(in0, in1, c0, c1, c2):
        return in0.astype(np.float32) ** 32

    ops = []
    for name, spec in [
        ("EXP32P_ANT", Spec(body=body1, reference=ref1)),
        ("POW32_ANT", Spec(body=body2, reference=ref2)),
    ]:
        op = D.DveOp(name, spec, subdim=False, uops_sha={})
        opc = D._CUSTOM_DVE_ROW_BASE + len(D.OPS)
        D.OPS.append(op)
        D._SUB_OPCODE_FOR_NAME[name] = opc
        D.CUSTOM_DVE_SPECS[name] = spec
        sha = DveOpSpec(
            name=name, opcode=opc, uops=lower(spec, ver="v3"),
            rd1_en=_has_src1(spec),
        ).sha("v3")
        op.uops_sha["v3"] = sha
        ops.append(op)
    return ops[0], ops[1]


def _build_nc():
    import concourse.bass as bass
    import concourse.mybir as mybir
    from concourse.tile import TileContext
    from concourse.vector_clock import ScopedClock

    class TileContextP(TileContext):
        """TileContext adapted to this walrus build, which only supports ONE
        sync-wait per instruction: extra waits are hoisted onto same-engine
        NoOps emitted immediately before the instruction."""

        def _commit_instruction(self, inst, lazy_reg_writes=True):
            si = getattr(inst, "sync_info", None)
            eng = getattr(inst, "engine", None)
            if si is not None and eng is not None:
                waits = list(si.on_wait or [])
                if len(waits) > 1:
                    for w in waits[:-1]:
                        nop = mybir.InstNoOp(
                            name=self.nc.get_next_instruction_name(),
                            sync_info=mybir.SyncInfo(on_wait=[w], on_update=[]),
                            bass_nofuse=True,
                            engine=eng,
                        )
                        super()._commit_instruction(nop, lazy_reg_writes=False)
                    si.on_wait = [waits[-1]]
            return super()._commit_instruction(inst, lazy_reg_writes)

        def _drain_and_barrier(self, tick_clock, wait_clock):
            nc = self.nc
            drain_inst = nc.sync.drain()
            wait_clock.add_sem_waits(
                drain_inst.ins, ScopedClock({None: tick_clock.global_clock})
            )
            si = drain_inst.ins.sync_info
            waits = list(si.on_wait or []) if si is not None else []
            if len(waits) > 1:
                si.on_wait = [waits[0]]
                # distribute remaining waits round-robin across engines so
                # they resolve in parallel rather than serially on SP
                engs = [nc.vector, nc.scalar, nc.tensor, nc.gpsimd, nc.sync]
                for j, w in enumerate(waits[1:]):
                    d = engs[j % len(engs)].drain()
                    d.ins.sync_info = mybir.SyncInfo(on_wait=[w], on_update=[])
            nc.all_engine_barrier(sem_only=True)
            assert self.sems is not None
            popped = nc._tile_sem_poison_stack.pop()
            assert popped is self._sem_poison
            nc.clear_and_free_semaphores(list(self.sems.allocated().values()))
            nc.all_engine_barrier(sem_only=True)

    f32 = mybir.dt.float32
    bf16 = mybir.dt.bfloat16

    nc = bass.Bass("TRN2")
    qt = nc.dram_tensor("qt", [NKV, HD, QG], bf16, kind="ExternalInput")
    kt = nc.dram_tensor("kt", [NKV, HD, K_TOT], bf16, kind="ExternalInput")
    va = nc.dram_tensor("va", [NKV, KT_TILES, 128, HD + 1], bf16,
                        kind="ExternalInput")
    mk = nc.dram_tensor("mk", [128, QG], bf16, kind="ExternalInput")
    o = nc.dram_tensor("o", [LQ, NH * HD], f32, kind="ExternalOutput")

    # k-tiles are exp'd in triples: one [128, 3*QG] PSUM chunk spans three
    # banks, cutting the per-ACTIVATE fixed cost. 17 tiles -> 5 triples + 1
    # pair. PSUM: scores 3 banks x 2 bufs + 2 packed accumulator banks = 8.
    chunks = [(3 * i, 3) for i in range(5)] + [(15, 2)]

    with TileContextP(nc) as tc:
        with (
            tc.tile_pool(name="kp", bufs=2) as kp,
            tc.tile_pool(name="vp", bufs=2) as vp,
            tc.tile_pool(name="qp", bufs=2) as qp,
            tc.tile_pool(name="mp", bufs=1) as mp,
            tc.tile_pool(name="pp", bufs=3) as pp,
            tc.tile_pool(name="rp", bufs=8) as rp,
            tc.tile_pool(name="ob", bufs=2) as ob,
            tc.tile_pool(name="sp", bufs=2, space="PSUM") as sp,
            tc.tile_pool(name="op", bufs=2, space="PSUM") as op,
        ):
            # warm the ACT exp table while the first DMAs are in flight
            warm = mp.tile([128, 1], mybir.dt.float32, name="warm")
            nc.vector.memset(warm, 0.0)
            nc.scalar.activation(warm, warm,
                                 mybir.ActivationFunctionType.Exp)

            mask_sb = mp.tile([128, QG], bf16)
            nc.sync.dma_start(out=mask_sb, in_=mk[:, :])



            # Flat software pipeline over (group, chunk): each chunk's PV
            # matmuls are emitted AFTER the next chunk's QK matmuls so the
            # in-order PE stream never head-of-line blocks on exp results or
            # on the previous group's accumulator release.
            accs_of = {}
            qt_of, kt_of, va_of = {}, {}, {}

            def load_group(g):
                qt_sb = qp.tile([HD, QG], bf16, tag="qt", name=f"qt{g}")
                nc.sync.dma_start(out=qt_sb, in_=qt[g])
                kt_sb = kp.tile([HD, K_TOT], bf16, tag="kt", name=f"kt{g}")
                # split so the first chunk's K columns land early
                nc.sync.dma_start(out=kt_sb[:, :3 * 128],
                                  in_=kt[g][:, :3 * 128])
                nc.sync.dma_start(out=kt_sb[:, 3 * 128:],
                                  in_=kt[g][:, 3 * 128:])
                va_sb = vp.tile([128, KT_TILES, HD + 1], bf16, tag="va",
                                name=f"va{g}")
                nc.sync.dma_start(
                    out=va_sb, in_=va[g].rearrange("t p c -> p t c"))
                kt_of[g], va_of[g], qt_of[g] = kt_sb, va_sb, qt_sb
                # two heads' [q, d+1] accumulators packed per PSUM bank
                accs_of[g] = [
                    op.tile([LQ, 2, HD + 1], mybir.dt.float32, tag="acc",
                            name=f"acc_{g}_{p}")
                    for p in range(GROUP // 2)
                ]

            def emit_qk_exp(g, t0, width):
                s_ps = sp.tile([128, 3 * QG], mybir.dt.float32, tag="s",
                               name=f"s_{g}_{t0}")
                for tt in range(width):
                    nc.tensor.matmul(
                        s_ps[:, tt * QG:(tt + 1) * QG],
                        lhsT=kt_of[g][:, (t0 + tt) * 128:(t0 + tt + 1) * 128],
                        rhs=qt_of[g],
                        start=True,
                        stop=True,
                    )
                p_sb = pp.tile([128, 3 * QG], bf16, tag="p",
                               name=f"p_{g}_{t0}")
                nc.scalar.activation(
                    p_sb[:, :width * QG], s_ps[:, :width * QG],
                    mybir.ActivationFunctionType.Exp, scale=SCALE,
                )
                if t0 + width == KT_TILES:
                    # new-token tile: multiplicative block-causal mask applied
                    # post-exp on DVE (keeps ACT's critical path mask-free)
                    lo = (width - 1) * QG
                    nc.vector.tensor_mul(
                        p_sb[:, lo:lo + QG], p_sb[:, lo:lo + QG], mask_sb)
                return p_sb

            def emit_pv(g, t0, width, p_sb):
                for tt in range(width):
                    t = t0 + tt
                    for h in range(GROUP):
                        # start=True clears has_written for the WHOLE bank, so
                        # only the first head sharing the bank issues it; the
                        # second head's t=0 write lands on has_written=0 and
                        # overwrites rather than accumulates.
                        nc.tensor.matmul(
                            accs_of[g][h // 2][:, h % 2, :],
                            lhsT=p_sb[:, tt * QG + h * LQ:
                                      tt * QG + (h + 1) * LQ],
                            rhs=va_of[g][:, t, :],
                            start=(t == 0 and h % 2 == 0),
                            stop=(t == KT_TILES - 1),
                            skip_group_check=True,
                        )

            def emit_normalize(g):
                o_sb = ob.tile([128, GROUP * HD], f32, tag="osb",
                               name=f"osb{g}")
                for h in range(GROUP):
                    acc = accs_of[g][h // 2][:, h % 2, :]
                    rec = rp.tile([LQ, 1], mybir.dt.float32, tag="rec",
                                  name=f"rec_{g}_{h}")
                    nc.vector.reciprocal(rec, acc[:, HD:HD + 1])
                    nc.vector.tensor_scalar_mul(
                        o_sb[:, h * HD:(h + 1) * HD],
                        acc[:, 0:HD],
                        rec,
                    )
                nc.sync.dma_start(
                    out=o[:, g * GROUP * HD:(g + 1) * GROUP * HD], in_=o_sb)

            work = [(g, t0, w) for g in range(NKV) for t0, w in chunks]
            load_group(0)
            pending = None  # (g, t0, width, p_sb) awaiting PV emission
            for g, t0, w in work:
                if t0 == 0 and g + 1 < NKV:
                    load_group(g + 1)
                p_sb = emit_qk_exp(g, t0, w)
                if pending is not None:
                    emit_pv(*pending)
                    if pending[1] + pending[2] == KT_TILES:
                        emit_normalize(pending[0])
                pending = (g, t0, w, p_sb)
            emit_pv(*pending)
            emit_normalize(pending[0])
    return nc


def _prep_inputs(q, k, v, k_cache, v_cache, page_tables, ctx_len, block_size):
    ctx = int(ctx_len)
    bs = int(block_size)
    assert ctx == CTX, f"kernel compiled for ctx_len={CTX}, got {ctx}"
    npages = ctx // PAGE

    q = np.asarray(q, np.float32).reshape(B, LQ, NH, HD)
    k = np.asarray(k, np.float32).reshape(B, LQ, NKV, HD)
    v = np.asarray(v, np.float32).reshape(B, LQ, NKV, HD)
    k_cache = np.asarray(k_cache, np.float32)
    v_cache = np.asarray(v_cache, np.float32)
    pt = np.asarray(page_tables).astype(np.int64)[:, :npages]

    # paged gather: [B, ctx, NKV, HD]
    k_ctx = k_cache[pt].reshape(B, ctx, NKV, HD)
    v_ctx = v_cache[pt].reshape(B, ctx, NKV, HD)
    k_full = np.concatenate([k_ctx, k], axis=1)   # [B, K_TOT, NKV, HD]
    v_full = np.concatenate([v_ctx, v], axis=1)

    # K^T per core: [NKV, HD, K_TOT], bf16
    kt = np.ascontiguousarray(
        k_full.transpose(0, 2, 3, 1)).astype(ml_dtypes.bfloat16)
    # V augmented with ones column, bf16: [B, NKV, K_TOT, HD+1]
    v_t = v_full.transpose(0, 2, 1, 3)            # [B, NKV, K_TOT, HD]
    va = np.empty((B, NKV, K_TOT, HD + 1), np.float32)
    va[..., :HD] = v_t
    va[..., HD] = 1.0
    va = va.astype(ml_dtypes.bfloat16).reshape(B, NKV, KT_TILES, 128, HD + 1)
    # Q^T per group: [B, NKV, HD, GROUP*LQ]
    qh = q.transpose(0, 2, 3, 1).reshape(B, NKV, GROUP, HD, LQ)
    qt = np.ascontiguousarray(qh.transpose(0, 1, 3, 2, 4)).reshape(
        B, NKV, HD, QG).astype(ml_dtypes.bfloat16)

    # multiplicative 0/1 mask on the new-token k-tile, scoresT coords [k, q4]
    kj = np.arange(128)
    qi = np.arange(LQ)
    allowed = (kj[:, None] // bs) <= (qi[None, :] // bs)   # [128, LQ]
    mrow = allowed.astype(np.float32)
    mask = np.tile(mrow, (1, GROUP)).astype(ml_dtypes.bfloat16)  # [128, QG]

    in_maps = []
    for b in range(B):
        in_maps.append({
            "qt": qt[b],
            "kt": kt[b],
            "va": va[b],
            "mk": mask,
        })
    return in_maps


def _run(inputs, trace=False):
    from concourse.bass_utils import run_bass_kernel_spmd

    if "nc" not in _CACHE:
        _CACHE["nc"] = _build_nc()
    nc = _CACHE["nc"]
    in_maps = _prep_inputs(**inputs)
    res = run_bass_kernel_spmd(
        nc, in_maps, core_ids=list(range(B)), trace=trace,
    )
    out = np.empty((B * LQ, NH * HD), np.float32)
    for b in range(B):
        out[b * LQ:(b + 1) * LQ] = res.results[b]["o"]
    return out, res


def kernel(**inputs):
    out, _ = _run(inputs, trace=False)
    return out


# revision 38
# speedup vs baseline: 1.3844x; 1.0796x over previous
"""Paged GQA attention (diffusion block-causal) on 8 TRN2 NeuronCores.

Problem: B=8 seqs x LQ=128 new tokens, 32 q heads / 8 kv heads, head_dim 128,
ctx_len=2048 cached tokens per seq (paged KV cache, 16-token pages), plus the
128 new tokens; block-causal mask (block 32) over the new-token region.

Sharding: one sequence per NeuronCore (8 seqs -> 8 cores), no collectives.

Per-core device kernel, per kv-head group g (4 q heads share a kv head):
  scoresT[k, q4] = K_g^T-tile.T @ Q_g          (fp32r matmul, N=512 full rate)
  probsT = exp(scoresT * scale + mask)         (ACT, bf16 out; mask on last tile)
  outU[q, d+1] += probsT_head.T @ [V_g | 1]    (bf16 matmul; last col = sum)
  out[q, d] = outU[:, :d] * (1 / outU[:, d])   (DVE reciprocal + tensor_scalar)

Host side: gather pages, build K^T / Q^T / V-augmented layouts, shard, gather.
"""

import sys

if '/opt/trn_rl_repo' not in sys.path:
    sys.path.insert(0, '/opt/trn_rl_repo')

import math

import ml_dtypes
import numpy as np

B = 8
LQ = 128
NH = 32
NKV = 8
GROUP = NH // NKV  # 4
HD = 128
PAGE = 16
CTX = 2048
K_TOT = CTX + LQ          # 2176
KT_TILES = K_TOT // 128   # 17
QG = GROUP * LQ           # 512 (4 heads x 128 queries)
SCALE = 1.0 / math.sqrt(HD)
MASK_NEG = -30000.0

_CACHE = {}


def _register_dve_exp_ops():
    """Two chained custom DVE ops computing exp(x*SCALE) at line rate on the
    Vector engine: p = deg-3 Taylor of e^(x*SCALE/32), then p^32 via five
    squarings. Lets DVE take a share of the softmax exp off the Scalar
    engine (the bottleneck)."""
    import numpy as np
    from concourse import dve_ops as D
    from concourse.dve_spec import (
        C0, C1, C2, One, Spec, Src0, _has_src1, lower, sq,
    )
    from concourse.dve_uop import DveOpSpec

    if "EXP32P_ANT" in D._SUB_OPCODE_FOR_NAME:
        by_name = {op.name: op for op in D.OPS}
        return by_name["EXP32P_ANT"], by_name["POW32_ANT"]

    v = Src0 * C0
    v2 = v * v
    body1 = (One + v) + v2 * (C1 + v * C2)

    def ref1(in0, in1, c0, c1, c2):
        vv = in0.astype(np.float32) * c0
        return (1.0 + vv + vv * vv * (c1 + vv * c2)).astype(np.float32)

    body2 = sq(sq(sq(sq(sq(Src0)))))

    def ref2(in0, in1, c0, c1, c2):
        return in0.astype(np.float32) ** 32

    ops = []
    for name, spec in [
        ("EXP32P_ANT", Spec(body=body1, reference=ref1)),
        ("POW32_ANT", Spec(body=body2, reference=ref2)),
    ]:
        op = D.DveOp(name, spec, subdim=False, uops_sha={})
        opc = D._CUSTOM_DVE_ROW_BASE + len(D.OPS)
        D.OPS.append(op)
        D._SUB_OPCODE_FOR_NAME[name] = opc
        D.CUSTOM_DVE_SPECS[name] = spec
        sha = DveOpSpec(
            name=name, opcode=opc, uops=lower(spec, ver="v3"),
            rd1_en=_has_src1(spec),
        ).sha("v3")
        op.uops_sha["v3"] = sha
        ops.append(op)
    return ops[0], ops[1]


def _build_nc():
    import concourse.bass as bass
    import concourse.mybir as mybir
    from concourse.tile import TileContext
    from concourse.vector_clock import ScopedClock

    class TileContextP(TileContext):
        """TileContext adapted to this walrus build, which only supports ONE
        sync-wait per instruction: extra waits are hoisted onto same-engine
        NoOps emitted immediately before the instruction."""

        def _commit_instruction(self, inst, lazy_reg_writes=True):
            si = getattr(inst, "sync_info", None)
            eng = getattr(inst, "engine", None)
            if si is not None and eng is not None:
                waits = list(si.on_wait or [])
                if len(waits) > 1:
                    for w in waits[:-1]:
                        nop = mybir.InstNoOp(
                            name=self.nc.get_next_instruction_name(),
                            sync_info=mybir.SyncInfo(on_wait=[w], on_update=[]),
                            bass_nofuse=True,
                            engine=eng,
                        )
                        super()._commit_instruction(nop, lazy_reg_writes=False)
                    si.on_wait = [waits[-1]]
            return super()._commit_instruction(inst, lazy_reg_writes)

        def _drain_and_barrier(self, tick_clock, wait_clock):
            nc = self.nc
            drain_inst = nc.sync.drain()
            wait_clock.add_sem_waits(
                drain_inst.ins, ScopedClock({None: tick_clock.global_clock})
            )
            si = drain_inst.ins.sync_info
            waits = list(si.on_wait or []) if si is not None else []
            if len(waits) > 1:
                si.on_wait = [waits[0]]
                # distribute remaining waits round-robin across engines so
                # they resolve in parallel rather than serially on SP
                engs = [nc.vector, nc.scalar, nc.tensor, nc.gpsimd, nc.sync]
                for j, w in enumerate(waits[1:]):
                    d = engs[j % len(engs)].drain()
                    d.ins.sync_info = mybir.SyncInfo(on_wait=[w], on_update=[])
            nc.all_engine_barrier(sem_only=True)
            assert self.sems is not None
            popped = nc._tile_sem_poison_stack.pop()
            assert popped is self._sem_poison
            nc.clear_and_free_semaphores(list(self.sems.allocated().values()))
            nc.all_engine_barrier(sem_only=True)

    f32 = mybir.dt.float32
    bf16 = mybir.dt.bfloat16

    nc = bass.Bass("TRN2")
    qt = nc.dram_tensor("qt", [NKV, HD, QG], bf16, kind="ExternalInput")
    kt = nc.dram_tensor("kt", [NKV, HD, K_TOT], bf16, kind="ExternalInput")
    va = nc.dram_tensor("va", [NKV, KT_TILES, 128, HD + 1], bf16,
                        kind="ExternalInput")
    mk = nc.dram_tensor("mk", [128, QG], bf16, kind="ExternalInput")
    o = nc.dram_tensor("o", [LQ, NH * HD], f32, kind="ExternalOutput")

    # k-tiles are exp'd in triples: one [128, 3*QG] PSUM chunk spans three
    # banks, cutting the per-ACTIVATE fixed cost. 17 tiles -> 5 triples + 1
    # pair. PSUM: scores 3 banks x 2 bufs + 2 packed accumulator banks = 8.
    chunks = [(3 * i, 3) for i in range(5)] + [(15, 2)]

    with TileContextP(nc) as tc:
        with (
            tc.tile_pool(name="kp", bufs=2) as kp,
            tc.tile_pool(name="vp", bufs=2) as vp,
            tc.tile_pool(name="qp", bufs=2) as qp,
            tc.tile_pool(name="mp", bufs=1) as mp,
            tc.tile_pool(name="pp", bufs=4) as pp,
            tc.tile_pool(name="rp", bufs=2) as rp,
            tc.tile_pool(name="ob", bufs=2) as ob,
            tc.tile_pool(name="sp", bufs=2, space="PSUM") as sp,
            tc.tile_pool(name="op", bufs=2, space="PSUM") as op,
        ):
            # warm the ACT exp table while the first DMAs are in flight
            warm = mp.tile([128, 1], mybir.dt.float32, name="warm")
            nc.vector.memset(warm, 0.0)
            nc.scalar.activation(warm, warm,
                                 mybir.ActivationFunctionType.Exp)

            mask_sb = mp.tile([128, QG], bf16)
            nc.sync.dma_start(out=mask_sb, in_=mk[:, :])



            # Flat software pipeline over (group, chunk): each chunk's PV
            # matmuls are emitted AFTER the next chunk's QK matmuls so the
            # in-order PE stream never head-of-line blocks on exp results or
            # on the previous group's accumulator release.
            accs_of = {}
            qt_of, kt_of, va_of = {}, {}, {}

            def load_group(g):
                qt_sb = qp.tile([HD, QG], bf16, tag="qt", name=f"qt{g}")
                nc.sync.dma_start(out=qt_sb, in_=qt[g])
                kt_sb = kp.tile([HD, K_TOT], bf16, tag="kt", name=f"kt{g}")
                # split so the first chunk's K columns land early
                nc.sync.dma_start(out=kt_sb[:, :3 * 128],
                                  in_=kt[g][:, :3 * 128])
                nc.sync.dma_start(out=kt_sb[:, 3 * 128:],
                                  in_=kt[g][:, 3 * 128:])
                va_sb = vp.tile([128, KT_TILES, HD + 1], bf16, tag="va",
                                name=f"va{g}")
                nc.sync.dma_start(
                    out=va_sb, in_=va[g].rearrange("t p c -> p t c"))
                kt_of[g], va_of[g], qt_of[g] = kt_sb, va_sb, qt_sb
                # two heads' [q, d+1] accumulators packed per PSUM bank
                accs_of[g] = [
                    op.tile([LQ, 2, HD + 1], mybir.dt.float32, tag="acc",
                            name=f"acc_{g}_{p}")
                    for p in range(GROUP // 2)
                ]

            def emit_qk_exp(g, t0, width):
                s_ps = sp.tile([128, 3 * QG], mybir.dt.float32, tag="s",
                               name=f"s_{g}_{t0}")
                for tt in range(width):
                    nc.tensor.matmul(
                        s_ps[:, tt * QG:(tt + 1) * QG],
                        lhsT=kt_of[g][:, (t0 + tt) * 128:(t0 + tt + 1) * 128],
                        rhs=qt_of[g],
                        start=True,
                        stop=True,
                    )
                p_sb = pp.tile([128, 3 * QG], bf16, tag="p",
                               name=f"p_{g}_{t0}")
                nc.scalar.activation(
                    p_sb[:, :width * QG], s_ps[:, :width * QG],
                    mybir.ActivationFunctionType.Exp, scale=SCALE,
                )
                if t0 + width == KT_TILES:
                    # new-token tile: multiplicative block-causal mask applied
                    # post-exp on DVE (keeps ACT's critical path mask-free)
                    lo = (width - 1) * QG
                    nc.vector.tensor_mul(
                        p_sb[:, lo:lo + QG], p_sb[:, lo:lo + QG], mask_sb)
                return p_sb

            def emit_pv(g, t0, width, p_sb):
                for tt in range(width):
                    t = t0 + tt
                    for h in range(GROUP):
                        # start=True clears has_written for the WHOLE bank, so
                        # only the first head sharing the bank issues it; the
                        # second head's t=0 write lands on has_written=0 and
                        # overwrites rather than accumulates.
                        nc.tensor.matmul(
                            accs_of[g][h // 2][:, h % 2, :],
                            lhsT=p_sb[:, tt * QG + h * LQ:
                                      tt * QG + (h + 1) * LQ],
                            rhs=va_of[g][:, t, :],
                            start=(t == 0 and h % 2 == 0),
                            stop=(t == KT_TILES - 1),
                            skip_group_check=True,
                        )

            def emit_normalize(g):
                o_sb = ob.tile([128, GROUP * HD], f32, tag="osb",
                               name=f"osb{g}")
                for h in range(GROUP):
                    acc = accs_of[g][h // 2][:, h % 2, :]
                    rec = rp.tile([LQ, 1], mybir.dt.float32, tag="rec",
                                  name=f"rec_{g}_{h}")
                    nc.vector.reciprocal(rec, acc[:, HD:HD + 1])
                    nc.vector.tensor_scalar_mul(
                        o_sb[:, h * HD:(h + 1) * HD],
                        acc[:, 0:HD],
                        rec,
                    )
                nc.sync.dma_start(
                    out=o[:, g * GROUP * HD:(g + 1) * GROUP * HD], in_=o_sb)

            work = [(g, t0, w) for g in range(NKV) for t0, w in chunks]
            load_group(0)
            pending = []  # (g, t0, width, p_sb) awaiting PV emission
            PIPE = 2  # PV trails QK/exp by this many chunks
            for g, t0, w in work:
                if t0 == 0 and g + 1 < NKV:
                    load_group(g + 1)
                p_sb = emit_qk_exp(g, t0, w)
                pending.append((g, t0, w, p_sb))
                if len(pending) > PIPE:
                    ent = pending.pop(0)
                    emit_pv(*ent)
                    if ent[1] + ent[2] == KT_TILES:
                        emit_normalize(ent[0])
            for ent in pending:
                emit_pv(*ent)
                if ent[1] + ent[2] == KT_TILES:
                    emit_normalize(ent[0])
    return nc


def _prep_inputs(q, k, v, k_cache, v_cache, page_tables, ctx_len, block_size):
    ctx = int(ctx_len)
    bs = int(block_size)
    assert ctx == CTX, f"kernel compiled for ctx_len={CTX}, got {ctx}"
    npages = ctx // PAGE

    q = np.asarray(q, np.float32).reshape(B, LQ, NH, HD)
    k = np.asarray(k, np.float32).reshape(B, LQ, NKV, HD)
    v = np.asarray(v, np.float32).reshape(B, LQ, NKV, HD)
    k_cache = np.asarray(k_cache, np.float32)
    v_cache = np.asarray(v_cache, np.float32)
    pt = np.asarray(page_tables).astype(np.int64)[:, :npages]

    # paged gather: [B, ctx, NKV, HD]
    k_ctx = k_cache[pt].reshape(B, ctx, NKV, HD)
    v_ctx = v_cache[pt].reshape(B, ctx, NKV, HD)
    k_full = np.concatenate([k_ctx, k], axis=1)   # [B, K_TOT, NKV, HD]
    v_full = np.concatenate([v_ctx, v], axis=1)

    # K^T per core: [NKV, HD, K_TOT], bf16
    kt = np.ascontiguousarray(
        k_full.transpose(0, 2, 3, 1)).astype(ml_dtypes.bfloat16)
    # V augmented with ones column, bf16: [B, NKV, K_TOT, HD+1]
    v_t = v_full.transpose(0, 2, 1, 3)            # [B, NKV, K_TOT, HD]
    va = np.empty((B, NKV, K_TOT, HD + 1), np.float32)
    va[..., :HD] = v_t
    va[..., HD] = 1.0
    va = va.astype(ml_dtypes.bfloat16).reshape(B, NKV, KT_TILES, 128, HD + 1)
    # Q^T per group: [B, NKV, HD, GROUP*LQ]
    qh = q.transpose(0, 2, 3, 1).reshape(B, NKV, GROUP, HD, LQ)
    qt = np.ascontiguousarray(qh.transpose(0, 1, 3, 2, 4)).reshape(
        B, NKV, HD, QG).astype(ml_dtypes.bfloat16)

    # multiplicative 0/1 mask on the new-token k-tile, scoresT coords [k, q4]
    kj = np.arange(128)
    qi = np.arange(LQ)
    allowed = (kj[:, None] // bs) <= (qi[None, :] // bs)   # [128, LQ]
    mrow = allowed.astype(np.float32)
    mask = np.tile(mrow, (1, GROUP)).astype(ml_dtypes.bfloat16)  # [128, QG]

    in_maps = []
    for b in range(B):
        in_maps.append({
            "qt": qt[b],
            "kt": kt[b],
            "va": va[b],
            "mk": mask,
        })
    return in_maps


def _run(inputs, trace=False):
    from concourse.bass_utils import run_bass_kernel_spmd

    if "nc" not in _CACHE:
        _CACHE["nc"] = _build_nc()
    nc = _CACHE["nc"]
    in_maps = _prep_inputs(**inputs)
    res = run_bass_kernel_spmd(
        nc, in_maps, core_ids=list(range(B)), trace=trace,
    )
    out = np.empty((B * LQ, NH * HD), np.float32)
    for b in range(B):
        out[b * LQ:(b + 1) * LQ] = res.results[b]["o"]
    return out, res


def kernel(**inputs):
    out, _ = _run(inputs, trace=False)
    return out


# revision 39
# speedup vs baseline: 1.4326x; 1.0348x over previous
"""Paged GQA attention (diffusion block-causal) on 8 TRN2 NeuronCores.

Problem: B=8 seqs x LQ=128 new tokens, 32 q heads / 8 kv heads, head_dim 128,
ctx_len=2048 cached tokens per seq (paged KV cache, 16-token pages), plus the
128 new tokens; block-causal mask (block 32) over the new-token region.

Sharding: one sequence per NeuronCore (8 seqs -> 8 cores), no collectives.

Per-core device kernel, per kv-head group g (4 q heads share a kv head):
  scoresT[k, q4] = K_g^T-tile.T @ Q_g          (fp32r matmul, N=512 full rate)
  probsT = exp(scoresT * scale + mask)         (ACT, bf16 out; mask on last tile)
  outU[q, d+1] += probsT_head.T @ [V_g | 1]    (bf16 matmul; last col = sum)
  out[q, d] = outU[:, :d] * (1 / outU[:, d])   (DVE reciprocal + tensor_scalar)

Host side: gather pages, build K^T / Q^T / V-augmented layouts, shard, gather.
"""

import sys

if '/opt/trn_rl_repo' not in sys.path:
    sys.path.insert(0, '/opt/trn_rl_repo')

import math

import ml_dtypes
import numpy as np

B = 8
LQ = 128
NH = 32
NKV = 8
GROUP = NH // NKV  # 4
HD = 128
PAGE = 16
CTX = 2048
K_TOT = CTX + LQ          # 2176
KT_TILES = K_TOT // 128   # 17
QG = GROUP * LQ           # 512 (4 heads x 128 queries)
SCALE = 1.0 / math.sqrt(HD)
MASK_NEG = -30000.0

_CACHE = {}


def _register_dve_exp_ops():
    """Two chained custom DVE ops computing exp(x*SCALE) at line rate on the
    Vector engine: p = deg-3 Taylor of e^(x*SCALE/32), then p^32 via five
    squarings. Lets DVE take a share of the softmax exp off the Scalar
    engine (the bottleneck)."""
    import numpy as np
    from concourse import dve_ops as D
    from concourse.dve_spec import (
        C0, C1, C2, One, Spec, Src0, _has_src1, lower, sq,
    )
    from concourse.dve_uop import DveOpSpec

    if "EXP32P_ANT" in D._SUB_OPCODE_FOR_NAME:
        by_name = {op.name: op for op in D.OPS}
        return by_name["EXP32P_ANT"], by_name["POW32_ANT"]

    v = Src0 * C0
    v2 = v * v
    body1 = (One + v) + v2 * (C1 + v * C2)

    def ref1(in0, in1, c0, c1, c2):
        vv = in0.astype(np.float32) * c0
        return (1.0 + vv + vv * vv * (c1 + vv * c2)).astype(np.float32)

    body2 = sq(sq(sq(sq(sq(Src0)))))

    def ref2(in0, in1, c0, c1, c2):
        return in0.astype(np.float32) ** 32

    ops = []
    for name, spec in [
        ("EXP32P_ANT", Spec(body=body1, reference=ref1)),
        ("POW32_ANT", Spec(body=body2, reference=ref2)),
    ]:
        op = D.DveOp(name, spec, subdim=False, uops_sha={})
        opc = D._CUSTOM_DVE_ROW_BASE + len(D.OPS)
        D.OPS.append(op)
        D._SUB_OPCODE_FOR_NAME[name] = opc
        D.CUSTOM_DVE_SPECS[name] = spec
        sha = DveOpSpec(
            name=name, opcode=opc, uops=lower(spec, ver="v3"),
            rd1_en=_has_src1(spec),
        ).sha("v3")
        op.uops_sha["v3"] = sha
        ops.append(op)
    return ops[0], ops[1]


def _build_nc():
    import concourse.bass as bass
    import concourse.mybir as mybir
    from concourse.tile import TileContext
    from concourse.vector_clock import ScopedClock

    class TileContextP(TileContext):
        """TileContext adapted to this walrus build, which only supports ONE
        sync-wait per instruction: extra waits are hoisted onto same-engine
        NoOps emitted immediately before the instruction."""

        def _commit_instruction(self, inst, lazy_reg_writes=True):
            si = getattr(inst, "sync_info", None)
            eng = getattr(inst, "engine", None)
            if si is not None and eng is not None:
                waits = list(si.on_wait or [])
                if len(waits) > 1:
                    for w in waits[:-1]:
                        nop = mybir.InstNoOp(
                            name=self.nc.get_next_instruction_name(),
                            sync_info=mybir.SyncInfo(on_wait=[w], on_update=[]),
                            bass_nofuse=True,
                            engine=eng,
                        )
                        super()._commit_instruction(nop, lazy_reg_writes=False)
                    si.on_wait = [waits[-1]]
            return super()._commit_instruction(inst, lazy_reg_writes)

        def _drain_and_barrier(self, tick_clock, wait_clock):
            nc = self.nc
            drain_inst = nc.sync.drain()
            wait_clock.add_sem_waits(
                drain_inst.ins, ScopedClock({None: tick_clock.global_clock})
            )
            si = drain_inst.ins.sync_info
            waits = list(si.on_wait or []) if si is not None else []
            if len(waits) > 1:
                si.on_wait = [waits[0]]
                # distribute remaining waits round-robin across engines so
                # they resolve in parallel rather than serially on SP
                engs = [nc.vector, nc.scalar, nc.tensor, nc.gpsimd, nc.sync]
                for j, w in enumerate(waits[1:]):
                    d = engs[j % len(engs)].drain()
                    d.ins.sync_info = mybir.SyncInfo(on_wait=[w], on_update=[])
            nc.all_engine_barrier(sem_only=True)
            assert self.sems is not None
            popped = nc._tile_sem_poison_stack.pop()
            assert popped is self._sem_poison
            nc.clear_and_free_semaphores(list(self.sems.allocated().values()))
            nc.all_engine_barrier(sem_only=True)

    f32 = mybir.dt.float32
    bf16 = mybir.dt.bfloat16

    nc = bass.Bass("TRN2")
    qt = nc.dram_tensor("qt", [NKV, HD, QG], bf16, kind="ExternalInput")
    kt = nc.dram_tensor("kt", [NKV, HD, K_TOT], bf16, kind="ExternalInput")
    va = nc.dram_tensor("va", [NKV, KT_TILES, 128, HD + 1], bf16,
                        kind="ExternalInput")
    mk = nc.dram_tensor("mk", [128, QG], bf16, kind="ExternalInput")
    o = nc.dram_tensor("o", [LQ, NH * HD], f32, kind="ExternalOutput")

    # k-tiles are exp'd in triples: one [128, 3*QG] PSUM chunk spans three
    # banks, cutting the per-ACTIVATE fixed cost. 17 tiles -> 5 triples + 1
    # pair. PSUM: scores 3 banks x 2 bufs + 2 packed accumulator banks = 8.
    chunks = [(3 * i, 3) for i in range(5)] + [(15, 2)]

    with TileContextP(nc) as tc:
        with (
            tc.tile_pool(name="kp", bufs=2) as kp,
            tc.tile_pool(name="vp", bufs=2) as vp,
            tc.tile_pool(name="qp", bufs=2) as qp,
            tc.tile_pool(name="mp", bufs=1) as mp,
            tc.tile_pool(name="pp", bufs=5) as pp,
            tc.tile_pool(name="rp", bufs=2) as rp,
            tc.tile_pool(name="ob", bufs=2) as ob,
            tc.tile_pool(name="sp", bufs=2, space="PSUM") as sp,
            tc.tile_pool(name="op", bufs=2, space="PSUM") as op,
        ):
            # warm the ACT exp table while the first DMAs are in flight
            warm = mp.tile([128, 1], mybir.dt.float32, name="warm")
            nc.vector.memset(warm, 0.0)
            nc.scalar.activation(warm, warm,
                                 mybir.ActivationFunctionType.Exp)

            mask_sb = mp.tile([128, QG], bf16)
            nc.sync.dma_start(out=mask_sb, in_=mk[:, :])



            # Flat software pipeline over (group, chunk): each chunk's PV
            # matmuls are emitted AFTER the next chunk's QK matmuls so the
            # in-order PE stream never head-of-line blocks on exp results or
            # on the previous group's accumulator release.
            accs_of = {}
            qt_of, kt_of, va_of = {}, {}, {}

            def load_group(g):
                qt_sb = qp.tile([HD, QG], bf16, tag="qt", name=f"qt{g}")
                nc.sync.dma_start(out=qt_sb, in_=qt[g])
                kt_sb = kp.tile([HD, K_TOT], bf16, tag="kt", name=f"kt{g}")
                # split so the first chunk's K columns land early
                nc.sync.dma_start(out=kt_sb[:, :3 * 128],
                                  in_=kt[g][:, :3 * 128])
                nc.sync.dma_start(out=kt_sb[:, 3 * 128:],
                                  in_=kt[g][:, 3 * 128:])
                va_sb = vp.tile([128, KT_TILES, HD + 1], bf16, tag="va",
                                name=f"va{g}")
                nc.sync.dma_start(
                    out=va_sb, in_=va[g].rearrange("t p c -> p t c"))
                kt_of[g], va_of[g], qt_of[g] = kt_sb, va_sb, qt_sb
                # two heads' [q, d+1] accumulators packed per PSUM bank
                accs_of[g] = [
                    op.tile([LQ, 2, HD + 1], mybir.dt.float32, tag="acc",
                            name=f"acc_{g}_{p}")
                    for p in range(GROUP // 2)
                ]

            def emit_qk_exp(g, t0, width):
                s_ps = sp.tile([128, 3 * QG], mybir.dt.float32, tag="s",
                               name=f"s_{g}_{t0}")
                for tt in range(width):
                    nc.tensor.matmul(
                        s_ps[:, tt * QG:(tt + 1) * QG],
                        lhsT=kt_of[g][:, (t0 + tt) * 128:(t0 + tt + 1) * 128],
                        rhs=qt_of[g],
                        start=True,
                        stop=True,
                    )
                p_sb = pp.tile([128, 3 * QG], bf16, tag="p",
                               name=f"p_{g}_{t0}")
                nc.scalar.activation(
                    p_sb[:, :width * QG], s_ps[:, :width * QG],
                    mybir.ActivationFunctionType.Exp, scale=SCALE,
                )
                if t0 + width == KT_TILES:
                    # new-token tile: multiplicative block-causal mask applied
                    # post-exp on DVE (keeps ACT's critical path mask-free)
                    lo = (width - 1) * QG
                    nc.vector.tensor_mul(
                        p_sb[:, lo:lo + QG], p_sb[:, lo:lo + QG], mask_sb)
                return p_sb

            def emit_pv(g, t0, width, p_sb):
                for tt in range(width):
                    t = t0 + tt
                    for h in range(GROUP):
                        # start=True clears has_written for the WHOLE bank, so
                        # only the first head sharing the bank issues it; the
                        # second head's t=0 write lands on has_written=0 and
                        # overwrites rather than accumulates.
                        nc.tensor.matmul(
                            accs_of[g][h // 2][:, h % 2, :],
                            lhsT=p_sb[:, tt * QG + h * LQ:
                                      tt * QG + (h + 1) * LQ],
                            rhs=va_of[g][:, t, :],
                            start=(t == 0 and h % 2 == 0),
                            stop=(t == KT_TILES - 1),
                            skip_group_check=True,
                        )

            def emit_normalize(g):
                o_sb = ob.tile([128, GROUP * HD], f32, tag="osb",
                               name=f"osb{g}")
                for h in range(GROUP):
                    acc = accs_of[g][h // 2][:, h % 2, :]
                    rec = rp.tile([LQ, 1], mybir.dt.float32, tag="rec",
                                  name=f"rec_{g}_{h}")
                    nc.vector.reciprocal(rec, acc[:, HD:HD + 1])
                    nc.vector.tensor_scalar_mul(
                        o_sb[:, h * HD:(h + 1) * HD],
                        acc[:, 0:HD],
                        rec,
                    )
                nc.sync.dma_start(
                    out=o[:, g * GROUP * HD:(g + 1) * GROUP * HD], in_=o_sb)

            work = [(g, t0, w) for g in range(NKV) for t0, w in chunks]
            load_group(0)
            pending = []  # (g, t0, width, p_sb) awaiting PV emission
            PIPE = 3  # PV trails QK/exp by this many chunks
            for g, t0, w in work:
                if t0 == 0 and g + 1 < NKV:
                    load_group(g + 1)
                p_sb = emit_qk_exp(g, t0, w)
                pending.append((g, t0, w, p_sb))
                if len(pending) > PIPE:
                    ent = pending.pop(0)
                    emit_pv(*ent)
                    if ent[1] + ent[2] == KT_TILES:
                        emit_normalize(ent[0])
            for ent in pending:
                emit_pv(*ent)
                if ent[1] + ent[2] == KT_TILES:
                    emit_normalize(ent[0])
    return nc


def _prep_inputs(q, k, v, k_cache, v_cache, page_tables, ctx_len, block_size):
    ctx = int(ctx_len)
    bs = int(block_size)
    assert ctx == CTX, f"kernel compiled for ctx_len={CTX}, got {ctx}"
    npages = ctx // PAGE

    q = np.asarray(q, np.float32).reshape(B, LQ, NH, HD)
    k = np.asarray(k, np.float32).reshape(B, LQ, NKV, HD)
    v = np.asarray(v, np.float32).reshape(B, LQ, NKV, HD)
    k_cache = np.asarray(k_cache, np.float32)
    v_cache = np.asarray(v_cache, np.float32)
    pt = np.asarray(page_tables).astype(np.int64)[:, :npages]

    # paged gather: [B, ctx, NKV, HD]
    k_ctx = k_cache[pt].reshape(B, ctx, NKV, HD)
    v_ctx = v_cache[pt].reshape(B, ctx, NKV, HD)
    k_full = np.concatenate([k_ctx, k], axis=1)   # [B, K_TOT, NKV, HD]
    v_full = np.concatenate([v_ctx, v], axis=1)

    # K^T per core: [NKV, HD, K_TOT], bf16
    kt = np.ascontiguousarray(
        k_full.transpose(0, 2, 3, 1)).astype(ml_dtypes.bfloat16)
    # V augmented with ones column, bf16: [B, NKV, K_TOT, HD+1]
    v_t = v_full.transpose(0, 2, 1, 3)            # [B, NKV, K_TOT, HD]
    va = np.empty((B, NKV, K_TOT, HD + 1), np.float32)
    va[..., :HD] = v_t
    va[..., HD] = 1.0
    va = va.astype(ml_dtypes.bfloat16).reshape(B, NKV, KT_TILES, 128, HD + 1)
    # Q^T per group: [B, NKV, HD, GROUP*LQ]
    qh = q.transpose(0, 2, 3, 1).reshape(B, NKV, GROUP, HD, LQ)
    qt = np.ascontiguousarray(qh.transpose(0, 1, 3, 2, 4)).reshape(
        B, NKV, HD, QG).astype(ml_dtypes.bfloat16)

    # multiplicative 0/1 mask on the new-token k-tile, scoresT coords [k, q4]
    kj = np.arange(128)
    qi = np.arange(LQ)
    allowed = (kj[:, None] // bs) <= (qi[None, :] // bs)   # [128, LQ]
    mrow = allowed.astype(np.float32)
    mask = np.tile(mrow, (1, GROUP)).astype(ml_dtypes.bfloat16)  # [128, QG]

    in_maps = []
    for b in range(B):
        in_maps.append({
            "qt": qt[b],
            "kt": kt[b],
            "va": va[b],
            "mk": mask,
        })
    return in_maps


def _run(inputs, trace=False):
    from concourse.bass_utils import run_bass_kernel_spmd

    if "nc" not in _CACHE:
        _CACHE["nc"] = _build_nc()
    nc = _CACHE["nc"]
    in_maps = _prep_inputs(**inputs)
    res = run_bass_kernel_spmd(
        nc, in_maps, core_ids=list(range(B)), trace=trace,
    )
    out = np.empty((B * LQ, NH * HD), np.float32)
    for b in range(B):
        out[b * LQ:(b + 1) * LQ] = res.results[b]["o"]
    return out, res


def kernel(**inputs):
    out, _ = _run(inputs, trace=False)
    return out
